# revision 23
# baseline (speedup 1.0000x reference)
"""MultiHead GQA (16 q heads / 4 kv heads, E=1024, n=2048, b=1) on 8 TRN2 cores.

Sharding: 256 query tokens per core; K/V projections replicated (cheaper than
any collective at this size).  All matmuls bf16 with fp32 PSUM accumulation.

Device schedule (v2): a single wave stream over (p, st) where p indexes the 4
head-pairs-of-pairs and st the 16 key tiles.  Each wave issues the lo/hi S
matmuls back-to-back into two adjacent PSUM banks — the 64-row tiles
(tile_position (0,0)/(64,0)) execute concurrently on the PE — then one
batched exp ACTIVATE over both banks, then (lag-1) the two V-stationary O
matmuls of the previous wave.  Projection matmuls are drip-fed as fillers so
the PE never waits on the scalar engine's exp stream.  Attention output is
moved to token-major via [80,128] X-bar DMA transposes issued on the scalar
HWDGE ring (input loads own the sync ring).

RoPE in the reference is the identity for b=1 (seq index = batch index = 0,
so cos=1 / sin=0 exactly); it is therefore omitted.
"""

import numpy as np
import ml_dtypes

import concourse.bass as bass
import concourse.bacc as bacc
import concourse.tile as tile
from concourse import mybir
from concourse import bass_utils

F32 = mybir.dt.float32
BF16 = mybir.dt.bfloat16
AF = mybir.ActivationFunctionType
ALU = mybir.AluOpType

N_CORES = 8
E = 1024
QH = 16
KVH = 4
HD = 64
KVE = KVH * HD
SEQ = 2048
T = SEQ // N_CORES   # 256 query tokens per core
ST = SEQ // 128      # 16 key s-tiles
ET = E // 128        # 8 contraction e-tiles
EPS = 1e-5
SCALE = 1.0 / (HD ** 0.5)
VP = 80              # padded hd+1 stride for V_sb (mult of 16 for transposes)

# Head bookkeeping: q-proj channel-tile t packs head EH[t] in partitions 0-63
# and head OH[t] in partitions 64-127.  EH heads use even kv heads (0, 2),
# OH heads use odd kv heads (1, 3), which matches the natural K-proj layout
# (K channel-tile 0 = kv0|kv1, tile 1 = kv2|kv3) with no partition shifts.
EH = [0, 1, 2, 3, 8, 9, 10, 11]
OH = [4, 5, 6, 7, 12, 13, 14, 15]
KV_LO = [EH[2 * p] // 4 for p in range(4)]   # [0, 0, 2, 2]
KV_HI = [OH[2 * p] // 4 for p in range(4)]   # [1, 1, 3, 3]

COLPERM = np.concatenate(
    [np.r_[EH[t] * HD:(EH[t] + 1) * HD, OH[t] * HD:(OH[t] + 1) * HD]
     for t in range(8)])

_CACHE = {}


def _build(apply_gb=True):
    nc = bacc.Bacc("TRN2", target_bir_lowering=False, debug=False)

    # DRAM layouts mirror the SBUF tile layouts exactly (partition-major,
    # host pre-arranged) so every load is a few large contiguous descriptors.
    qT_d = nc.dram_tensor("qT", [128, ET, T], BF16, kind="ExternalInput").ap()
    kT_d = nc.dram_tensor("kT", [128, ET, SEQ], BF16, kind="ExternalInput").ap()
    vT_d = nc.dram_tensor("vT", [128, ET, SEQ], BF16, kind="ExternalInput").ap()
    wqA_d = nc.dram_tensor("wqA", [128, ET, 512], BF16, kind="ExternalInput").ap()
    wqB_d = nc.dram_tensor("wqB", [128, ET, 512], BF16, kind="ExternalInput").ap()
    wkT_d = nc.dram_tensor("wkT", [128, ET, KVE], BF16, kind="ExternalInput").ap()
    wvT_d = nc.dram_tensor("wvT", [128, ET, KVE], BF16, kind="ExternalInput").ap()
    bq_d = nc.dram_tensor("bq", [128, 8], F32, kind="ExternalInput").ap()
    bk_d = nc.dram_tensor("bk", [128, 2], F32, kind="ExternalInput").ap()
    bv_d = nc.dram_tensor("bv", [KVE], F32, kind="ExternalInput").ap()
    gam_d = nc.dram_tensor("gam", [E], F32, kind="ExternalInput").ap()
    bet_d = nc.dram_tensor("bet", [E], F32, kind="ExternalInput").ap()
    out_d = nc.dram_tensor("out", [T, E], F32, kind="ExternalOutput").ap()

    def bcast_row(dram_ap, n):
        return bass.AP(tensor=dram_ap.tensor, offset=0, ap=[[0, n]] + dram_ap.ap)

    with tile.TileContext(nc) as tc:
        with tc.tile_pool(name="persist", bufs=1) as P:
            # persistent SBUF tiles
            queryT = P.tile([128, ET, T], BF16)
            wqT = P.tile([128, ET, E], BF16)
            wkT = P.tile([128, ET, KVE], BF16)
            keyT = P.tile([128, ET, SEQ], BF16)
            wvT = P.tile([128, ET, KVE], BF16)
            valueT = P.tile([128, ET, SEQ], BF16)
            bq_s = P.tile([128, 8], F32)
            bk_s = P.tile([128, 2], F32)
            bvB = P.tile([128, KVE], F32)
            gamB = P.tile([128, E], F32)
            betB = P.tile([128, E], F32)

            q_sb = P.tile([128, 8, T], BF16)
            K_sb = P.tile([128, 2, SEQ], BF16)
            V_sb = P.tile([128, KVH, ST, VP], BF16)
            OUT = P.tile([128, 2, E], F32)
            STATS = P.tile([128, 2, QH, 6], F32)
            eps_t = P.tile([128, 1], F32)

            # Input loads: all on the sync HWDGE ring, strictly in
            # consumption order (FIFO ring -> arrival order == issue order).
            # keyT is split so kproj(0,0) unlocks after ~1 MB.
            nc.sync.dma_start(out=bk_s, in_=bk_d)
            nc.sync.dma_start(out=bq_s, in_=bq_d)
            nc.sync.dma_start(out=bvB, in_=bcast_row(bv_d, 128))
            nc.sync.dma_start(out=wkT, in_=wkT_d)
            nc.sync.dma_start(out=keyT[:, :, 0:512], in_=kT_d[:, :, 0:512])
            nc.sync.dma_start(out=queryT, in_=qT_d)
            nc.sync.dma_start(out=wqT[:, :, 0:512], in_=wqA_d)
            nc.sync.dma_start(out=keyT[:, :, 512:1024], in_=kT_d[:, :, 512:1024])
            nc.sync.dma_start(out=wvT, in_=wvT_d)
            nc.sync.dma_start(out=valueT[:, :, 0:512], in_=vT_d[:, :, 0:512])
            nc.sync.dma_start(out=valueT[:, :, 512:1024], in_=vT_d[:, :, 512:1024])
            nc.sync.dma_start(out=keyT[:, :, 1024:2048], in_=kT_d[:, :, 1024:2048])
            nc.sync.dma_start(out=valueT[:, :, 1024:2048], in_=vT_d[:, :, 1024:2048])
            nc.sync.dma_start(out=wqT[:, :, 512:1024], in_=wqB_d)
            nc.sync.dma_start(out=gamB, in_=bcast_row(gam_d, 128))
            nc.sync.dma_start(out=betB, in_=bcast_row(bet_d, 128))

            nc.vector.memset(eps_t, EPS)
            nc.vector.memset(V_sb[:, :, :, HD:HD + 1], 1.0)
            junk = P.tile([128, 512], BF16)
            nc.vector.memset(junk, 0.0)

            with tc.tile_pool(name="sw", bufs=1, space="PSUM") as swp, \
                 tc.tile_pool(name="op", bufs=1, space="PSUM") as opp, \
                 tc.tile_pool(name="pp", bufs=1, space="PSUM") as ppp, \
                 tc.tile_pool(name="ering", bufs=8) as erp, \
                 tc.tile_pool(name="tail", bufs=4) as tlp, \
                 tc.tile_pool(name="tail8", bufs=8) as tl8:

                # ---------------- projection units (fillers) ----------------
                # Each unit is a list of thunks; each thunk emits ONE matmul
                # (or the evacuation op).  A global scheduler drips them into
                # the wave stream strictly in `order`.
                def kproj_unit(c, j, tg):
                    pk = [None]

                    def mk(e):
                        def f():
                            if pk[0] is None:
                                pk[0] = ppp.tile([128, 512], F32, tag=f"pp{tg}",
                                                 name=f"pk{c}{j}")
                            nc.tensor.matmul(
                                pk[0], wkT[:, e, 128 * c:128 * (c + 1)],
                                keyT[:, e, 512 * j:512 * (j + 1)],
                                start=(e == 0), stop=(e == ET - 1))
                        return f

                    def ev():
                        nc.vector.tensor_scalar_add(
                            out=K_sb[:, c, 512 * j:512 * (j + 1)], in0=pk[0],
                            scalar1=bk_s[:, c:c + 1])
                    return [mk(e) for e in range(ET)] + [ev]

                def qproj_unit(t, tg):
                    pq = [None]

                    def mk(e):
                        def f():
                            if pq[0] is None:
                                pq[0] = ppp.tile([128, 512], F32, tag=f"pp{tg}",
                                                 name=f"pq{t}")
                            nc.tensor.matmul(
                                pq[0][:, 0:T], wqT[:, e, 128 * t:128 * (t + 1)],
                                queryT[:, e, :], start=(e == 0), stop=(e == ET - 1))
                        return f

                    def ev():
                        nc.vector.tensor_scalar_add(
                            out=q_sb[:, t, :], in0=pq[0][:, 0:T],
                            scalar1=bq_s[:, t:t + 1])
                    return [mk(e) for e in range(ET)] + [ev]

                def vproj_unit(st, tg):
                    pv = [None]

                    def mk(e):
                        def f():
                            if pv[0] is None:
                                pv[0] = ppp.tile([128, 512], F32, tag=f"pp{tg}",
                                                 name=f"pv{st}")
                            nc.tensor.matmul(
                                pv[0][:, 0:KVE],
                                valueT[:, e, 128 * st:128 * (st + 1)],
                                wvT[:, e, :], start=(e == 0), stop=(e == ET - 1))
                        return f

                    def ev():
                        nc.vector.tensor_add(
                            out=V_sb[:, :, st, 0:HD],
                            in0=pv[0][:, 0:KVE].rearrange("p (h d) -> p h d", h=KVH),
                            in1=bvB.rearrange("p (h d) -> p h d", h=KVH))
                    return [mk(e) for e in range(ET)] + [ev]

                # unit order: q0 q1 (first compute), kp(0,0), then the rest
                # interleaved so every wave's deps are emitted slightly ahead
                # of use, and DMA-independent units (k1x, q4-7) fill early
                # bubbles while valueT/keyT chunks are still in flight.
                UNIT_ORDER = [
                    "k00", "q0", "q1", "v0", "k01", "v1", "k10", "v2", "q2",
                    "v3", "k11", "v4", "k02", "v5", "q3", "v6", "k12", "v7",
                    "k03", "v8", "q4", "v9", "k13", "v10", "q5", "v11", "q6",
                    "v12", "q7", "v13", "v14", "v15",
                ]
                units = {}
                for i, name in enumerate(UNIT_ORDER):
                    tg = i % 2
                    if name[0] == "q":
                        steps = qproj_unit(int(name[1]), tg)
                    elif name[0] == "k":
                        steps = kproj_unit(int(name[1]), int(name[2]), tg)
                    else:
                        steps = vproj_unit(int(name[1:]), tg)
                    units[name] = {"steps": steps, "i": 0}
                uidx = {name: i for i, name in enumerate(UNIT_ORDER)}
                cursor = [0]

                def run_steps(n):
                    # emit up to n filler steps, strictly in UNIT_ORDER
                    while n > 0 and cursor[0] < len(UNIT_ORDER):
                        u = units[UNIT_ORDER[cursor[0]]]
                        if u["i"] >= len(u["steps"]):
                            cursor[0] += 1
                            continue
                        u["steps"][u["i"]]()
                        u["i"] += 1
                        n -= 1

                def ensure(*names):
                    # emit all units up to and including every named unit
                    for name in names:
                        idx = uidx[name]
                        while cursor[0] <= idx:
                            u = units[UNIT_ORDER[cursor[0]]]
                            while u["i"] < len(u["steps"]):
                                u["steps"][u["i"]]()
                                u["i"] += 1
                            cursor[0] += 1

                # HAM warmup: PE activity from ~7us (right after the preamble)
                # so the clock gate is at 8/8 by the time real data arrives,
                # and stays warm through the first projections.
                wps = ppp.tile([128, 512], F32, tag="pp0", name="warm")
                for i in range(11):
                    nc.tensor.matmul(wps, junk[:, 0:128], junk,
                                     start=(i == 0), stop=(i == 10))

                # pre-stream: k00, q0, q1 fully emitted
                ensure("k00", "q0", "q1")

                # ---------------- wave stream ----------------
                def s_wave(p, st, widx):
                    c = KV_LO[p] // 2
                    sw = swp.tile([128, 2, 512], F32, tag=f"sw{widx % 2}",
                                  name=f"sw{p}_{st}")
                    nc.tensor.matmul(
                        sw[:, 0, :],
                        K_sb[0:64, c, 128 * st:128 * (st + 1)],
                        q_sb[0:64, 2 * p:2 * p + 2, :],
                        start=True, stop=True, tile_position=(0, 0))
                    nc.tensor.matmul(
                        sw[:, 1, :],
                        K_sb[64:128, c, 128 * st:128 * (st + 1)],
                        q_sb[64:128, 2 * p:2 * p + 2, :],
                        start=True, stop=True, tile_position=(64, 0))
                    ep = erp.tile([128, 2, 512], BF16, tag="e", name=f"ep{p}_{st}")
                    nc.scalar.activation(
                        out=ep.rearrange("p a b -> p (a b)"),
                        in_=sw.rearrange("p a b -> p (a b)"),
                        func=AF.Exp, scale=SCALE)
                    return ep

                o_tiles = {}

                def o_wave(p, st, ep):
                    for hi in (0, 1):
                        key = (p, hi)
                        if key not in o_tiles:
                            o_tiles[key] = opp.tile(
                                [128, 512], F32, tag="ohi" if hi else "olo",
                                name=f"o{p}_{hi}")
                        kv = KV_HI[p] if hi else KV_LO[p]
                        nc.tensor.matmul(
                            o_tiles[key][0:HD + 1, :],
                            V_sb[:, kv, st, 0:HD + 1],
                            ep[:, hi, :],
                            start=(st == 0), stop=(st == ST - 1))

                def evac(p, last=False):
                    # O psum -> bf16 staging (frees the psum banks for p+1).
                    # In the tail, split the two casts across vector + scalar.
                    st_tiles = {}
                    for hi in (0, 1):
                        o_ps = o_tiles.pop((p, hi))
                        o_st = tlp.tile([128, 512], BF16, tag=f"ost{hi}",
                                        name=f"ost{p}_{hi}")
                        nc.vector.memset(o_st[HD:VP, :], 0.0)
                        if last and hi:
                            nc.scalar.copy(out=o_st[0:HD + 1, :],
                                           in_=o_ps[0:HD + 1, :])
                        else:
                            nc.vector.tensor_copy(out=o_st[0:HD + 1, :],
                                                  in_=o_ps[0:HD + 1, :])
                        st_tiles[hi] = o_st
                    return st_tiles

                def finalize(p, st_tiles, last):
                    # transpose 4 [80,128] chunks per (p, hi) to token-major
                    # (sync HWDGE ring; the exp stream owns the scalar queue),
                    # then normalize by the softmax denominator + LN stats.
                    for hi in (0, 1):
                        o_st = st_tiles[hi]
                        for k in range(4):
                            tt, ch = k % 2, k // 2
                            h = (OH if hi else EH)[2 * p + ch]
                            ot = tl8.tile([128, VP], BF16, tag="ot",
                                          name=f"ot{p}_{hi}_{k}")
                            eng = nc.scalar if (last and k % 2) else nc.sync
                            eng.dma_start(out=ot,
                                          in_=o_st[0:VP, 128 * k:128 * (k + 1)],
                                          transpose=True)
                            rec = tl8.tile([128, 1], F32, tag="rec",
                                           name=f"rec{p}_{hi}_{k}")
                            nc.vector.reciprocal(out=rec, in_=ot[:, HD:HD + 1])
                            if last:
                                # offload the normalize to the (now idle)
                                # scalar engine; vector keeps recip + stats
                                nc.scalar.mul(out=OUT[:, tt, HD * h:HD * (h + 1)],
                                              in_=ot[:, 0:HD], mul=rec)
                            else:
                                nc.vector.tensor_scalar_mul(
                                    out=OUT[:, tt, HD * h:HD * (h + 1)],
                                    in0=ot[:, 0:HD], scalar1=rec)
                            nc.vector.bn_stats(
                                out=STATS[:, tt, h, :],
                                in_=OUT[:, tt, HD * h:HD * (h + 1)])

                # wave needs: kproj(c, st//4), q tiles 2p/2p+1 before S;
                # vproj(st) before (lag-1) O.
                def s_needs(p, st):
                    c = KV_LO[p] // 2
                    return (f"k{c}{st // 4}", f"q{2 * p}", f"q{2 * p + 1}")

                OLAG = 4      # waves between S/exp and the consuming O
                FLAG = 2      # extra waves before a finished p's transposes
                waves = [(p, st) for p in range(4) for st in range(ST)]
                opend = []    # (p, st, ep) waiting for O emission
                fpend = []    # (p, st_tiles, age) waiting for finalize
                for widx, (p, st) in enumerate(waves):
                    ensure(*s_needs(p, st))
                    ep = s_wave(p, st, widx)
                    opend.append((p, st, ep))
                    if len(opend) > OLAG:
                        pp_, pst, pep = opend.pop(0)
                        ensure(f"v{pst}")
                        o_wave(pp_, pst, pep)
                        if pst == ST - 1:
                            fpend.append([pp_, evac(pp_), 0])
                    if fpend:
                        fpend[0][2] += 1
                        if fpend[0][2] > FLAG:
                            fp, ftiles, _ = fpend.pop(0)
                            finalize(fp, ftiles, last=False)
                    run_steps(2)
                while opend:
                    pp_, pst, pep = opend.pop(0)
                    ensure(f"v{pst}")
                    o_wave(pp_, pst, pep)
                    if pst == ST - 1:
                        fpend.append([pp_, evac(pp_, last=(not opend)), 0])
                run_steps(10 ** 6)   # drain any remaining fillers
                while fpend:
                    fp, ftiles, _ = fpend.pop(0)
                    finalize(fp, ftiles, last=(not fpend))

                # ---------------- layernorm + store ----------------
                # tt=0 applied on vector, tt=1 on scalar (y = x*r - mu*r),
                # each DMA'd on its own HWDGE ring as soon as it is ready.
                mvs, rstds = [], []
                for tt in range(2):
                    mv = tlp.tile([128, 2], F32, tag=f"mv{tt}", name=f"mv{tt}")
                    nc.vector.bn_aggr(out=mv, in_=STATS[:, tt, :, :])
                    rstd = tlp.tile([128, 1], F32, tag=f"rstd{tt}",
                                    name=f"rstd{tt}")
                    nc.scalar.activation(out=rstd, in_=mv[:, 1:2], func=AF.Sqrt,
                                         bias=eps_t, scale=1.0)
                    nc.vector.reciprocal(out=rstd, in_=rstd)
                    mvs.append(mv)
                    rstds.append(rstd)
                y0 = tlp.tile([128, E], F32, tag="y0", name="y0")
                nc.vector.tensor_scalar(out=y0, in0=OUT[:, 0, :],
                                        scalar1=mvs[0][:, 0:1], scalar2=rstds[0],
                                        op0=ALU.subtract, op1=ALU.mult)
                if apply_gb:
                    nc.vector.tensor_mul(out=y0, in0=y0, in1=gamB)
                    nc.vector.tensor_add(out=y0, in0=y0, in1=betB)
                nc.sync.dma_start(out=out_d[0:128, :], in_=y0)
                nmur = tlp.tile([128, 1], F32, tag="nmur", name="nmur")
                nc.vector.tensor_scalar(out=nmur, in0=mvs[1][:, 0:1],
                                        scalar1=rstds[1], scalar2=-1.0,
                                        op0=ALU.mult, op1=ALU.mult)
                y1 = tlp.tile([128, E], F32, tag="y1", name="y1")
                nc.scalar.activation(out=y1, in_=OUT[:, 1, :], func=AF.Identity,
                                     scale=rstds[1], bias=nmur)
                if apply_gb:
                    nc.vector.tensor_mul(out=y1, in0=y1, in1=gamB)
                    nc.vector.tensor_add(out=y1, in0=y1, in1=betB)
                nc.scalar.dma_start(out=out_d[128:256, :], in_=y1)

    nc.compile()
    return nc


def _prep_inputs(query, key, value, Wq, bq, Wk, bk, Wv, bv, gamma, beta):
    bf = ml_dtypes.bfloat16
    query, key, value = np.asarray(query), np.asarray(key), np.asarray(value)
    Wq, Wk, Wv = np.asarray(Wq), np.asarray(Wk), np.asarray(Wv)
    bq, bk, bv = np.asarray(bq), np.asarray(bk), np.asarray(bv)
    def parr(x):
        # [E, N] -> [128, ET, N] partition-major (DRAM layout == SBUF layout)
        e, n = x.shape
        return np.ascontiguousarray(
            x.reshape(e // 128, 128, n).transpose(1, 0, 2).astype(bf))

    qT = parr(query[0].T)
    kT = parr(key[0].T)
    vT = parr(value[0].T)
    wq = parr(Wq.T[:, COLPERM])
    wqA = np.ascontiguousarray(wq[:, :, 0:512])
    wqB = np.ascontiguousarray(wq[:, :, 512:1024])
    wkT = parr(Wk.T)
    wvT = parr(Wv.T)
    bq_p = np.ascontiguousarray(bq[COLPERM].reshape(8, 128).T.astype(np.float32))
    bk_p = np.ascontiguousarray(bk.reshape(2, 128).T.astype(np.float32))
    common = {
        "kT": kT, "vT": vT, "wqA": wqA, "wqB": wqB, "wkT": wkT, "wvT": wvT,
        "bq": bq_p, "bk": bk_p, "bv": np.asarray(bv, np.float32),
        "gam": np.asarray(gamma, np.float32), "bet": np.asarray(beta, np.float32),
    }
    in_maps = []
    for c in range(N_CORES):
        m = dict(common)
        m["qT"] = np.ascontiguousarray(qT[:, :, T * c:T * (c + 1)])
        in_maps.append(m)
    return in_maps


def run(inputs, trace=False):
    trivial_gb = (np.all(np.asarray(inputs["gamma"]) == 1.0)
                  and np.all(np.asarray(inputs["beta"]) == 0.0))
    key = ("nc", not trivial_gb)
    if key not in _CACHE:
        _CACHE[key] = _build(apply_gb=not trivial_gb)
    nc = _CACHE[key]
    in_maps = _prep_inputs(**inputs)
    res = bass_utils.run_bass_kernel_spmd(
        nc, in_maps, core_ids=list(range(N_CORES)), trace=trace)
    out = np.empty((1, SEQ, E), np.float32)
    for c in range(N_CORES):
        out[0, T * c:T * (c + 1), :] = res.results[c]["out"]
    return out, res


def kernel(**inputs):
    out, _ = run(inputs, trace=False)
    return out
